# revision 43
# baseline (speedup 1.0000x reference)
"""Trainium2 Bass kernel for nn_BiLinearDotLayer.

Computes, for feature (B,F,E)=(2048,200,64) f32 and weight (F,E,E):
    bilinear[b,i,d] = sum_e feature[b,i,e] * weight[i,e,d]
    out[b,i,j]      = sum_d bilinear[b,i,d] * feature[b,j,d]

Strategy (8 NeuronCores, data-parallel over batch):
  - Each core handles 256 batches; weight replicated.
  - Host pre-transposes feature to featT[e, b, i] (fp16) and packs
    even/odd batches into SBUF partition halves (p*64+e) so all tiles
    use 128 partitions (full DMA width) and the two batch parities run
    concurrently on the PE array's row-strips (K=64 each).
  - Single resident block: the whole 256-batch shard + weights live in
    SBUF; both einsums run fully on-chip (fp16 operands, fp32 PSUM
    accumulation, ~5e-4 rel error); only the fp16 feature (6.5MB),
    fp16 weights (3.3MB) are read and the f32 output (41MB) written.
  - Output is written in a partition-contiguous device layout and
    un-permuted on the host.
"""

import os
import sys

for _p in ("/opt/trn_rl_repo", "/root/.axon_site/_ro/trn_rl_repo"):
    if os.path.isdir(_p) and _p not in sys.path:
        sys.path.insert(0, _p)

import numpy as np

B, F, E = 2048, 200, 64
NCORES = 8
BLOC = B // NCORES            # 256 batches per core
NPAIR = BLOC // 2             # 128 even/odd batch pairs per core
STG = 8                       # pairs per staged out-DMA (16 batches, 1.23MB)
# uniform 16-pair blocks: e1 of block k+1 interleaves with e2 stages of
# block k, so the out-DMA stream never goes dry during einsum1 work
E1_BLOCKS = [(i * 16, 16) for i in range(8)]  # (pair0, npairs)

_RUNNER = None


def _build_program():
    import concourse.tile as tile
    from concourse import bacc, mybir

    f32 = mybir.dt.float32
    fp16 = mybir.dt.float16
    nc = bacc.Bacc("TRN2", target_bir_lowering=False, debug=False)

    # feature packed as featT: fpk[p*64+e, bb*F+i] (fp16)
    fpk = nc.dram_tensor("fpk", [128, NPAIR * F], fp16, kind="ExternalInput")
    # weight packed as wpk[p*64+e, i*64+d] (fp16)
    wpk = nc.dram_tensor("wpk", [128, F * E], fp16, kind="ExternalInput")
    # Device-friendly output layout: out_dev[p, b, ci, j] = out[b, 2p+ci, j].
    # Each partition's slice is contiguous in DRAM; host un-permutes.
    out = nc.dram_tensor("out", [100, BLOC, 2, F], fp16, kind="ExternalOutput")
    out_v = out.ap()

    HP = NPAIR // 2  # pairs per half-shard

    with tile.TileContext(nc) as tc:
        with (
            tc.tile_pool(name="wpool", bufs=1) as wpool,
            tc.tile_pool(name="fpool", bufs=1) as fpool,
            tc.tile_pool(name="bpool", bufs=1) as bpool,
            tc.tile_pool(name="stpool", bufs=4) as stpool,
            tc.tile_pool(name="ps1", bufs=2, space="PSUM") as ps1pool,
            # einsum2 PSUM: one 2-bank tile per pair (both parities), so
            # the whole pair drains in a single copy. 3 bufs = 6 banks.
            tc.tile_pool(name="ps2", bufs=3, space="PSUM") as ps2pool,
        ):
            # whole-shard resident tiles
            ftile = fpool.tile([128, NPAIR * F], fp16, name="ftile", tag="ftile")
            # feature in growing pair-blocks on the sync ring; weights in
            # two halves on the scalar ring.
            wtile = wpool.tile([128, F * E], fp16, name="wtile", tag="wtile")
            wh = (F // 2) * E
            nc.scalar.dma_start(out=wtile[:, :wh], in_=wpk.ap()[:, :wh])
            nc.scalar.dma_start(out=wtile[:, wh:], in_=wpk.ap()[:, wh:])
            for _p0, _np in E1_BLOCKS:
                nc.sync.dma_start(
                    out=ftile[:, _p0 * F : (_p0 + _np) * F],
                    in_=fpk.ap()[:, _p0 * F : (_p0 + _np) * F],
                )
            # bilinear, i-major: btile[p*64+d, i*NPAIR + bb]
            btile = bpool.tile([128, NPAIR * F], fp16, name="btile", tag="btile")

            f3 = ftile[:].rearrange("p (bb i) -> p bb i", i=F)
            b3i = btile[:].rearrange("p (i bb) -> p i bb", bb=NPAIR)
            # lhsT view for einsum2: (p, ci, i2, bb) with i = 2*i2 + ci
            bt4 = btile[:].rearrange("p (i2 ci bb) -> p ci i2 bb", ci=2, bb=NPAIR)

            cpy = 0
            dma_i = 0

            def e1_block(p0, np_):
                """einsum1 for all i x pairs [p0, p0+np_)."""
                nonlocal cpy
                igrp = max(1, 512 // np_)
                for i0 in range(0, F, igrp):
                    gs = min(igrp, F - i0)
                    pst = ps1pool.tile([128, 512], f32, name="pst", tag="pst")
                    for g in range(gs):
                        i = i0 + g
                        for p in (0, 1):
                            pr = slice(p * 64, (p + 1) * 64)
                            nc.tensor.matmul(
                                out=pst[pr, g * np_ : (g + 1) * np_],
                                lhsT=wtile[pr, i * E : (i + 1) * E],
                                rhs=f3[pr, p0 : p0 + np_, i],
                                start=True,
                                stop=True,
                            )
                    src = pst[:, : gs * np_].rearrange(
                        "p (g bb) -> p g bb", bb=np_
                    )
                    dst = b3i[:, i0 : i0 + gs, p0 : p0 + np_]
                    # cast fp32 psum -> fp16 bilinear tile (60/40 DVE/ACT)
                    if cpy % 5 < 3:
                        nc.vector.tensor_copy(out=dst, in_=src)
                    else:
                        nc.scalar.copy(out=dst, in_=src)
                    cpy += 1

            def e2_stage_group(m):
                """einsum2 for pairs [m, m+STG) + staged out-DMA.

                i-chunks are stride-2 interleaved (ci = i%2) so out partition
                p owns rows i=2p, 2p+1 of each out[b]."""
                nonlocal cpy, dma_i
                stage = stpool.tile([128, STG * 4 * F], fp16, name="stage", tag="stage")
                for u in range(STG):
                    bb = m + u
                    # both parities of one pair in a 2-bank PSUM tile:
                    # (p, ci) segment at 256-aligned offset p*512 + ci*256
                    psAB = ps2pool.tile([128, 1024], f32, name="psAB", tag="psAB")
                    for ci in (0, 1):
                        for p in (0, 1):
                            pr = slice(p * 64, (p + 1) * 64)
                            nc.tensor.matmul(
                                out=psAB[
                                    0:100,
                                    p * 512 + ci * 256 : p * 512 + ci * 256 + F,
                                ],
                                lhsT=bt4[pr, ci, :, bb],
                                rhs=ftile[pr, bb * F : (bb + 1) * F],
                                start=True,
                                stop=True,
                            )
                    # one drain copy per pair (800 payload cols)
                    src = psAB[0:100].rearrange(
                        "q (b ci j) -> q b ci j", b=2, j=256
                    )[:, :, :, 0:F]
                    dst = stage[
                        0:100, u * 4 * F : (u + 1) * 4 * F
                    ].rearrange("q (b ci j) -> q b ci j", b=2, j=F)
                    # 60/40 DVE/ACT balance (DVE is ~1.45x faster here)
                    if cpy % 5 < 3:
                        nc.vector.tensor_copy(out=dst, in_=src)
                    else:
                        nc.scalar.copy(out=dst, in_=src)
                    cpy += 1
                b0 = 2 * m
                dma_eng = nc.sync if dma_i % 2 == 0 else nc.scalar
                rem_eng = nc.scalar if dma_i % 2 == 0 else nc.sync
                dma_i += 1
                # 96-partition store engages all 16 SDMA engines (engine
                # count = largest divisor of partition count <= 16); the
                # 4-partition remnant rides the other ring.
                sview = stage[:, :].rearrange("p (b ci j) -> p b ci j", ci=2, j=F)
                dview = out_v[:, b0 : b0 + 2 * STG, :, :]
                dma_eng.dma_start(out=dview[0:96], in_=sview[0:96])
                rem_eng.dma_start(out=dview[96:100], in_=sview[96:100])

            # Block-progressive schedule: e1 then e2 per feature block, so
            # the first out-DMAs issue as soon as block 0 is computed.
            for p0, np_ in E1_BLOCKS:
                e1_block(p0, np_)
                for m in range(p0, p0 + np_, STG):
                    e2_stage_group(m)

    nc.compile()
    return nc


class _Runner:
    """Builds the program once and keeps a reusable sharded jit."""

    def __init__(self):
        self.nc = _build_program()
        import jax
        from jax.sharding import Mesh, PartitionSpec
        from jax.experimental.shard_map import shard_map
        from concourse import mybir
        from concourse import bass2jax

        bass2jax.install_neuronx_cc_hook()
        nc = self.nc

        partition_name = (
            nc.partition_id_tensor.name if nc.partition_id_tensor else None
        )
        in_names, out_names, out_avals, zero_outs = [], [], [], []
        for alloc in nc.m.functions[0].allocations:
            if not isinstance(alloc, mybir.MemoryLocationSet):
                continue
            name = alloc.memorylocations[0].name
            if alloc.kind == "ExternalInput":
                if name != partition_name:
                    in_names.append(name)
            elif alloc.kind == "ExternalOutput":
                shape = tuple(alloc.tensor_shape)
                dtype = mybir.dt.np(alloc.dtype)
                out_names.append(name)
                out_avals.append(jax.core.ShapedArray(shape, dtype))
                zero_outs.append(np.zeros(shape, dtype))
        self.in_names = list(in_names)
        self.out_names = out_names
        self.out_avals = out_avals
        self.zero_outs = zero_outs
        n_params = len(in_names)
        n_outs = len(out_avals)
        in_names_full = in_names + out_names
        if partition_name is not None:
            in_names_full.append(partition_name)
        donate = tuple(range(n_params, n_params + n_outs))

        def _body(*args):
            operands = list(args)
            if partition_name is not None:
                operands.append(bass2jax.partition_id_tensor())
            outs = bass2jax._bass_exec_p.bind(
                *operands,
                out_avals=tuple(out_avals),
                in_names=tuple(in_names_full),
                out_names=tuple(out_names),
                lowering_input_output_aliases=(),
                sim_require_finite=True,
                sim_require_nnan=True,
                nc=nc,
            )
            return tuple(outs)

        devices = jax.devices()[:NCORES]
        mesh = Mesh(np.asarray(devices), ("core",))
        in_specs = (PartitionSpec("core"),) * (n_params + n_outs)
        out_specs = (PartitionSpec("core"),) * n_outs
        self.sharded = jax.jit(
            shard_map(
                _body,
                mesh=mesh,
                in_specs=in_specs,
                out_specs=out_specs,
                check_rep=False,
            ),
            donate_argnums=donate,
            keep_unused=True,
        )

    def run(self, concat_inputs):
        """concat_inputs: dict name -> (8*shape0, ...) array."""
        args = [concat_inputs[n] for n in self.in_names]
        zeros = [
            np.zeros((NCORES * z.shape[0], *z.shape[1:]), z.dtype)
            for z in self.zero_outs
        ]
        outs = self.sharded(*args, *zeros)
        return {n: np.asarray(outs[i]) for i, n in enumerate(self.out_names)}


def _get_runner():
    global _RUNNER
    if _RUNNER is None:
        _RUNNER = _Runner()
    return _RUNNER


def pack_inputs(feature, weight):
    """Host-side packing: returns dict of concatenated per-core inputs."""
    feature = np.ascontiguousarray(np.asarray(feature, dtype=np.float32))
    weight = np.ascontiguousarray(np.asarray(weight, dtype=np.float32))
    # featT pack: fpk[core][p*64+e, bb*F+i] = feature[core*BLOC + 2*bb + p, i, e]
    ft = feature.reshape(NCORES, NPAIR, 2, F, E)  # [core, bb, p, i, e]
    fpk = (
        np.ascontiguousarray(ft.transpose(0, 2, 4, 1, 3))
        .reshape(NCORES * 128, NPAIR * F)
        .astype(np.float16)
    )
    wt = np.ascontiguousarray(weight.transpose(1, 0, 2)).reshape(E, F * E)
    wpk_one = np.concatenate([wt, wt], axis=0).astype(np.float16)  # (128, F*E)
    wpk = np.tile(wpk_one, (NCORES, 1))
    return {"fpk": fpk, "wpk": wpk}


def kernel(feature, weight):
    r = _get_runner()
    ins = pack_inputs(feature, weight)
    outs = r.run(ins)
    return unpack_output(outs["out"])


def unpack_output(out_dev):
    """out_dev: (8*100, BLOC, 2, F) fp16 device layout -> (B, F, F) f32."""
    o = out_dev.reshape(NCORES, 100, BLOC, 2, F).astype(np.float32)
    # out[core, b, 2p+ci, j] = o[core, p, b, ci, j]
    return np.ascontiguousarray(o.transpose(0, 2, 1, 3, 4)).reshape(B, F, F)


if __name__ == "__main__":
    rng = np.random.default_rng(0)
    feature = rng.standard_normal((B, F, E), dtype=np.float32)
    weight = (0.01 * rng.standard_normal((F, E, E))).astype(np.float32)
    got = kernel(feature, weight)
    bil = np.einsum(
        "bie,ied->bid", feature.astype(np.float64), weight.astype(np.float64)
    )
    ref = np.einsum("bid,bjd->bij", bil, feature.astype(np.float64))
    err = np.abs(got - ref)
    denom = np.abs(ref).max()
    print("max abs err:", err.max(), "rel(scale):", err.max() / denom)
    l2 = np.linalg.norm((got - ref).ravel()) / np.linalg.norm(ref.ravel())
    print("L2 rel:", l2)



# revision 46
# speedup vs baseline: 1.1419x; 1.1419x over previous
"""Trainium2 Bass kernel for nn_BiLinearDotLayer.

Computes, for feature (B,F,E)=(2048,200,64) f32 and weight (F,E,E):
    bilinear[b,i,d] = sum_e feature[b,i,e] * weight[i,e,d]
    out[b,i,j]      = sum_d bilinear[b,i,d] * feature[b,j,d]

Strategy (8 NeuronCores, data-parallel over batch):
  - Each core handles 256 batches; weight replicated.
  - Host pre-transposes feature to featT[e, b, i] (fp16) and packs
    even/odd batches into SBUF partition halves (p*64+e) so all tiles
    use 128 partitions (full DMA width) and the two batch parities run
    concurrently on the PE array's row-strips (K=64 each).
  - Single resident block: the whole 256-batch shard + weights live in
    SBUF; both einsums run fully on-chip (fp16 operands, fp32 PSUM
    accumulation, ~5e-4 rel error); only the fp16 feature (6.5MB),
    fp16 weights (3.3MB) are read and the f32 output (41MB) written.
  - Output is written in a partition-contiguous device layout and
    un-permuted on the host.
"""

import os
import sys

for _p in ("/opt/trn_rl_repo", "/root/.axon_site/_ro/trn_rl_repo"):
    if os.path.isdir(_p) and _p not in sys.path:
        sys.path.insert(0, _p)

import numpy as np

B, F, E = 2048, 200, 64
NCORES = 8
BLOC = B // NCORES            # 256 batches per core
NPAIR = BLOC // 2             # 128 even/odd batch pairs per core
STG = 8                       # pairs per staged out-DMA (16 batches, 1.23MB)
E1_BLOCKS = [(0, 16), (16, 16), (32, 32), (64, 64)]  # (pair0, npairs)

_RUNNER = None


def _build_program():
    import concourse.tile as tile
    from concourse import bacc, mybir

    f32 = mybir.dt.float32
    fp16 = mybir.dt.float16
    nc = bacc.Bacc("TRN2", target_bir_lowering=False, debug=False)

    # feature packed as featT: fpk[p*64+e, bb*F+i] (fp16)
    fpk = nc.dram_tensor("fpk", [128, NPAIR * F], fp16, kind="ExternalInput")
    # weight packed as wpk[p*64+e, i*64+d] (fp16)
    wpk = nc.dram_tensor("wpk", [128, F * E], fp16, kind="ExternalInput")
    # Device-friendly output layout: out_dev[p, b, ci, j] = out[b, 2p+ci, j].
    # Each partition's slice is contiguous in DRAM; host un-permutes.
    out = nc.dram_tensor("out", [100, BLOC, 2, F], fp16, kind="ExternalOutput")
    out_v = out.ap()

    HP = NPAIR // 2  # pairs per half-shard

    with tile.TileContext(nc) as tc:
        with (
            tc.tile_pool(name="wpool", bufs=1) as wpool,
            tc.tile_pool(name="fpool", bufs=1) as fpool,
            tc.tile_pool(name="bpool", bufs=1) as bpool,
            tc.tile_pool(name="stpool", bufs=4) as stpool,
            tc.tile_pool(name="ps1", bufs=2, space="PSUM") as ps1pool,
            tc.tile_pool(name="ps2", bufs=3, space="PSUM") as ps2pool,
        ):
            # whole-shard resident tiles
            ftile = fpool.tile([128, NPAIR * F], fp16, name="ftile", tag="ftile")
            # feature in growing pair-blocks on the sync ring; weights in
            # two halves on the scalar ring.
            wtile = wpool.tile([128, F * E], fp16, name="wtile", tag="wtile")
            wh = (F // 2) * E
            nc.scalar.dma_start(out=wtile[:, :wh], in_=wpk.ap()[:, :wh])
            nc.scalar.dma_start(out=wtile[:, wh:], in_=wpk.ap()[:, wh:])
            for _p0, _np in E1_BLOCKS:
                nc.sync.dma_start(
                    out=ftile[:, _p0 * F : (_p0 + _np) * F],
                    in_=fpk.ap()[:, _p0 * F : (_p0 + _np) * F],
                )
            # bilinear, i-major: btile[p*64+d, i*NPAIR + bb]
            btile = bpool.tile([128, NPAIR * F], fp16, name="btile", tag="btile")

            f3 = ftile[:].rearrange("p (bb i) -> p bb i", i=F)
            b3i = btile[:].rearrange("p (i bb) -> p i bb", bb=NPAIR)
            # lhsT view for einsum2: (p, ci, i2, bb) with i = 2*i2 + ci
            bt4 = btile[:].rearrange("p (i2 ci bb) -> p ci i2 bb", ci=2, bb=NPAIR)

            cpy = 0
            dma_i = 0

            def e1_group(p0, np_, i0):
                """einsum1 for i in [i0, i0+igrp) x pairs [p0, p0+np_)."""
                nonlocal cpy
                igrp = max(1, 512 // np_)
                if True:
                    gs = min(igrp, F - i0)
                    pst = ps1pool.tile([128, 512], f32, name="pst", tag="pst")
                    for g in range(gs):
                        i = i0 + g
                        for p in (0, 1):
                            pr = slice(p * 64, (p + 1) * 64)
                            nc.tensor.matmul(
                                out=pst[pr, g * np_ : (g + 1) * np_],
                                lhsT=wtile[pr, i * E : (i + 1) * E],
                                rhs=f3[pr, p0 : p0 + np_, i],
                                start=True,
                                stop=True,
                            )
                    src = pst[:, : gs * np_].rearrange(
                        "p (g bb) -> p g bb", bb=np_
                    )
                    dst = b3i[:, i0 : i0 + gs, p0 : p0 + np_]
                    # cast fp32 psum -> fp16 bilinear tile (60/40 DVE/ACT)
                    if cpy % 5 < 3:
                        nc.vector.tensor_copy(out=dst, in_=src)
                    else:
                        nc.scalar.copy(out=dst, in_=src)
                    cpy += 1

            def e2_stage_group(m):
                """einsum2 for pairs [m, m+STG) + staged out-DMA.

                i-chunks are stride-2 interleaved (ci = i%2) so out partition
                p owns rows i=2p, 2p+1 of each out[b]."""
                nonlocal cpy, dma_i
                stage = stpool.tile([128, STG * 4 * F], fp16, name="stage", tag="stage")
                for u in range(STG):
                    bb = m + u
                    # both parities of one pair in a 2-bank PSUM tile:
                    # (p, ci) segment at 256-aligned offset p*512 + ci*256
                    psAB = ps2pool.tile([128, 1024], f32, name="psAB", tag="psAB")
                    for ci in (0, 1):
                        for p in (0, 1):
                            pr = slice(p * 64, (p + 1) * 64)
                            nc.tensor.matmul(
                                out=psAB[
                                    0:100,
                                    p * 512 + ci * 256 : p * 512 + ci * 256 + F,
                                ],
                                lhsT=bt4[pr, ci, :, bb],
                                rhs=ftile[pr, bb * F : (bb + 1) * F],
                                start=True,
                                stop=True,
                            )
                    # one drain copy per pair (800 payload cols), 60/40 DVE/ACT
                    csrc = psAB[0:100].rearrange(
                        "q (b ci j) -> q b ci j", b=2, j=256
                    )[:, :, :, 0:F]
                    cdst = stage[
                        0:100, u * 4 * F : (u + 1) * 4 * F
                    ].rearrange("q (b ci j) -> q b ci j", b=2, j=F)
                    if cpy % 5 < 3:
                        nc.vector.tensor_copy(out=cdst, in_=csrc)
                    else:
                        nc.scalar.copy(out=cdst, in_=csrc)
                    cpy += 1
                b0 = 2 * m
                dma_eng = nc.sync if dma_i % 2 == 0 else nc.scalar
                rem_eng = nc.scalar if dma_i % 2 == 0 else nc.sync
                dma_i += 1
                # 96-partition store engages all 16 SDMA engines (engine
                # count = largest divisor of partition count <= 16); the
                # 4-partition remnant rides the other ring.
                sview = stage[:, :].rearrange("p (b ci j) -> p b ci j", ci=2, j=F)
                dview = out_v[:, b0 : b0 + 2 * STG, :, :]
                dma_eng.dma_start(out=dview[0:96], in_=sview[0:96])
                rem_eng.dma_start(out=dview[96:100], in_=sview[96:100])

            # Block-progressive schedule with cross-block pacing: block k's
            # e2 stages are interleaved across block k+1's e1 groups so the
            # out-DMA stream never goes dry while einsum1 runs.
            prev_stages = []
            for p0, np_ in E1_BLOCKS:
                igrp = max(1, 512 // np_)
                groups = list(range(0, F, igrp))
                si = 0
                for gi, i0 in enumerate(groups):
                    e1_group(p0, np_, i0)
                    target = ((gi + 1) * len(prev_stages)) // len(groups)
                    while si < target:
                        e2_stage_group(prev_stages[si])
                        si += 1
                prev_stages = list(range(p0, p0 + np_, STG))
            for m in prev_stages:
                e2_stage_group(m)

    nc.compile()
    return nc


class _Runner:
    """Builds the program once and keeps a reusable sharded jit."""

    def __init__(self):
        self.nc = _build_program()
        import jax
        from jax.sharding import Mesh, PartitionSpec
        from jax.experimental.shard_map import shard_map
        from concourse import mybir
        from concourse import bass2jax

        bass2jax.install_neuronx_cc_hook()
        nc = self.nc

        partition_name = (
            nc.partition_id_tensor.name if nc.partition_id_tensor else None
        )
        in_names, out_names, out_avals, zero_outs = [], [], [], []
        for alloc in nc.m.functions[0].allocations:
            if not isinstance(alloc, mybir.MemoryLocationSet):
                continue
            name = alloc.memorylocations[0].name
            if alloc.kind == "ExternalInput":
                if name != partition_name:
                    in_names.append(name)
            elif alloc.kind == "ExternalOutput":
                shape = tuple(alloc.tensor_shape)
                dtype = mybir.dt.np(alloc.dtype)
                out_names.append(name)
                out_avals.append(jax.core.ShapedArray(shape, dtype))
                zero_outs.append(np.zeros(shape, dtype))
        self.in_names = list(in_names)
        self.out_names = out_names
        self.out_avals = out_avals
        self.zero_outs = zero_outs
        n_params = len(in_names)
        n_outs = len(out_avals)
        in_names_full = in_names + out_names
        if partition_name is not None:
            in_names_full.append(partition_name)
        donate = tuple(range(n_params, n_params + n_outs))

        def _body(*args):
            operands = list(args)
            if partition_name is not None:
                operands.append(bass2jax.partition_id_tensor())
            outs = bass2jax._bass_exec_p.bind(
                *operands,
                out_avals=tuple(out_avals),
                in_names=tuple(in_names_full),
                out_names=tuple(out_names),
                lowering_input_output_aliases=(),
                sim_require_finite=True,
                sim_require_nnan=True,
                nc=nc,
            )
            return tuple(outs)

        devices = jax.devices()[:NCORES]
        mesh = Mesh(np.asarray(devices), ("core",))
        in_specs = (PartitionSpec("core"),) * (n_params + n_outs)
        out_specs = (PartitionSpec("core"),) * n_outs
        self.sharded = jax.jit(
            shard_map(
                _body,
                mesh=mesh,
                in_specs=in_specs,
                out_specs=out_specs,
                check_rep=False,
            ),
            donate_argnums=donate,
            keep_unused=True,
        )

    def run(self, concat_inputs):
        """concat_inputs: dict name -> (8*shape0, ...) array."""
        args = [concat_inputs[n] for n in self.in_names]
        zeros = [
            np.zeros((NCORES * z.shape[0], *z.shape[1:]), z.dtype)
            for z in self.zero_outs
        ]
        outs = self.sharded(*args, *zeros)
        return {n: np.asarray(outs[i]) for i, n in enumerate(self.out_names)}


def _get_runner():
    global _RUNNER
    if _RUNNER is None:
        _RUNNER = _Runner()
    return _RUNNER


def pack_inputs(feature, weight):
    """Host-side packing: returns dict of concatenated per-core inputs."""
    feature = np.ascontiguousarray(np.asarray(feature, dtype=np.float32))
    weight = np.ascontiguousarray(np.asarray(weight, dtype=np.float32))
    # featT pack: fpk[core][p*64+e, bb*F+i] = feature[core*BLOC + 2*bb + p, i, e]
    ft = feature.reshape(NCORES, NPAIR, 2, F, E)  # [core, bb, p, i, e]
    fpk = (
        np.ascontiguousarray(ft.transpose(0, 2, 4, 1, 3))
        .reshape(NCORES * 128, NPAIR * F)
        .astype(np.float16)
    )
    wt = np.ascontiguousarray(weight.transpose(1, 0, 2)).reshape(E, F * E)
    wpk_one = np.concatenate([wt, wt], axis=0).astype(np.float16)  # (128, F*E)
    wpk = np.tile(wpk_one, (NCORES, 1))
    return {"fpk": fpk, "wpk": wpk}


def kernel(feature, weight):
    r = _get_runner()
    ins = pack_inputs(feature, weight)
    outs = r.run(ins)
    return unpack_output(outs["out"])


def unpack_output(out_dev):
    """out_dev: (8*100, BLOC, 2, F) fp16 device layout -> (B, F, F) f32."""
    o = out_dev.reshape(NCORES, 100, BLOC, 2, F).astype(np.float32)
    # out[core, b, 2p+ci, j] = o[core, p, b, ci, j]
    return np.ascontiguousarray(o.transpose(0, 2, 1, 3, 4)).reshape(B, F, F)


if __name__ == "__main__":
    rng = np.random.default_rng(0)
    feature = rng.standard_normal((B, F, E), dtype=np.float32)
    weight = (0.01 * rng.standard_normal((F, E, E))).astype(np.float32)
    got = kernel(feature, weight)
    bil = np.einsum(
        "bie,ied->bid", feature.astype(np.float64), weight.astype(np.float64)
    )
    ref = np.einsum("bid,bjd->bij", bil, feature.astype(np.float64))
    err = np.abs(got - ref)
    denom = np.abs(ref).max()
    print("max abs err:", err.max(), "rel(scale):", err.max() / denom)
    l2 = np.linalg.norm((got - ref).ravel()) / np.linalg.norm(ref.ravel())
    print("L2 rel:", l2)

